# revision 1
# baseline (speedup 1.0000x reference)
"""Fused attention kernel (nn_Attention_18708877541532) for 8 Trainium2 cores.

Strategy: data-parallel over batch B=16 -> 2 batches per core. Everything on
one core is computed in a "transposed" layout so no on-device transposes are
needed:
  - host passes x^T / lab^T (feature-major) in bf16
  - qT/kT = W^T @ x^T via PE  (inner on partitions)
  - scoresT[k, q] = kT_h-slice.T @ qT_h  (keys on partitions, both q-halves
    into one 2-bank PSUM tile)
  - exp via one ACT op per (head, key-chunk) with per-key bias (fused
    tanh-bias + mask) and 1/sqrt(d) scale
  - attendedT[d, q] = [v | 1]-chunks.T @ expT  -> row 64 = softmax sums
  - per-head-pair normalization with fast reciprocal + gpsimd partition
    broadcast (no end-of-batch sync)
  - O-projection consumes attendedT directly as lhsT; + bias + residual.
"""
import numpy as np
import ml_dtypes
from contextlib import ExitStack

import concourse.bass as bass
import concourse.tile as tile
from concourse import bacc, mybir
from concourse import bass_utils

B, QL, KL = 16, 1024, 512
EMBED, HEADS, DHEAD = 768, 12, 64
INNER = HEADS * DHEAD
NCORES = 8
BLOC = B // NCORES            # 2 batches per core
P = 128
EC = EMBED // P               # 6 embed chunks
MC = INNER // P               # 6 inner chunks
KC = KL // P                  # 4 key chunks
QH = 2                        # q halves
QW = QL // QH                 # 512
QT = QW // P                  # 4 q tiles per half
SCALE = float(DHEAD) ** -0.5

F32 = mybir.dt.float32
BF16 = mybir.dt.bfloat16
BF = ml_dtypes.bfloat16

_CACHE: dict = {}


def _build():
    nc = bacc.Bacc("TRN2", target_bir_lowering=False, debug=False,
                   enable_asserts=True, num_devices=NCORES)

    xT_d = nc.dram_tensor("xT", [BLOC, EMBED, QL], BF16, kind="ExternalInput").ap()
    labT_d = nc.dram_tensor("labT", [BLOC, EMBED, KL], BF16, kind="ExternalInput").ap()
    x_d = nc.dram_tensor("x", [BLOC, QL, EMBED], F32, kind="ExternalInput").ap()
    wq_d = nc.dram_tensor("Wq", [EMBED, INNER], BF16, kind="ExternalInput").ap()
    wk_d = nc.dram_tensor("Wk", [EMBED, INNER], BF16, kind="ExternalInput").ap()
    wv_d = nc.dram_tensor("Wv", [EMBED, INNER], BF16, kind="ExternalInput").ap()
    wo_d = nc.dram_tensor("Wo", [INNER, EMBED], BF16, kind="ExternalInput").ap()
    biask_d = nc.dram_tensor("biasK", [BLOC, KL], F32, kind="ExternalInput").ap()
    out_d = nc.dram_tensor("out", [BLOC, QL, EMBED], F32, kind="ExternalOutput").ap()

    with tile.TileContext(nc) as tc, ExitStack() as ctx:
        sb = ctx.enter_context(tc.tile_pool(name="sb", bufs=1))
        xtp = ctx.enter_context(tc.tile_pool(name="xtp", bufs=1))
        ltp = ctx.enter_context(tc.tile_pool(name="ltp", bufs=1))
        qtp = ctx.enter_context(tc.tile_pool(name="qtp", bufs=2))
        ktp = ctx.enter_context(tc.tile_pool(name="ktp", bufs=2))
        vtp = ctx.enter_context(tc.tile_pool(name="vtp", bufs=2))
        expp = ctx.enter_context(tc.tile_pool(name="expp", bufs=3))
        attp = ctx.enter_context(tc.tile_pool(name="attp", bufs=3))
        stp = ctx.enter_context(tc.tile_pool(name="stp", bufs=3))
        smp = ctx.enter_context(tc.tile_pool(name="smp", bufs=2))
        rcp = ctx.enter_context(tc.tile_pool(name="rcp", bufs=2))
        rsp = ctx.enter_context(tc.tile_pool(name="rsp", bufs=4))
        bcp = ctx.enter_context(tc.tile_pool(name="bcp", bufs=3))
        onp = ctx.enter_context(tc.tile_pool(name="onp", bufs=4))
        oup = ctx.enter_context(tc.tile_pool(name="oup", bufs=2))
        pp = ctx.enter_context(tc.tile_pool(name="pp", bufs=2, space="PSUM"))
        ps = ctx.enter_context(tc.tile_pool(name="ps", bufs=2, space="PSUM"))
        pa = ctx.enter_context(tc.tile_pool(name="pa", bufs=2, space="PSUM"))

        # ---- persistent tiles; chunked DMAs so compute can start early ----
        W = sb.tile([P, 4 * EC, INNER], BF16, tag="wall")
        wq_r = wq_d.rearrange("(c p) i -> p c i", p=P)
        wk_r = wk_d.rearrange("(c p) i -> p c i", p=P)
        wv_r = wv_d.rearrange("(c p) i -> p c i", p=P)
        wo_r = wo_d.rearrange("(c p) i -> p c i", p=P)

        biask_sb = sb.tile([P, BLOC, KC], F32, tag="biask")

        warm_bc = sb.tile([2, 8], BF16, tag="warmbc")

        def g_preload():
            xt0 = xtp.tile([P, EC, QL], BF16, tag="xT")
            xT_sb[0] = xt0
            xr0 = xT_d[0].rearrange("(c p) t -> p c t", p=P)
            for c in range(EC):
                nc.sync.dma_start(W[:, c, :], wq_r[:, c, :])
                nc.scalar.dma_start(xt0[:, c, 0:QW], xr0[:, c, 0:QW])
                nc.scalar.dma_start(xt0[:, c, QW:QL], xr0[:, c, QW:QL])
            for b in range(BLOC):
                nc.sync.dma_start(biask_sb[:, b, :],
                                  biask_d[b].rearrange("(c p) -> p c", p=P))
            # touch the custom-op library now so its ~7us IRAM load overlaps
            # the startup DMAs instead of stalling mid-kernel
            nc.vector.memset(warm_bc[0:1, :], 1.0)
            nc.gpsimd.partition_broadcast(warm_bc[:], warm_bc[0:1, :])
            yield
            for c in range(EC):
                nc.sync.dma_start(W[:, EC + c, :], wk_r[:, c, :])
            yield
            for c in range(EC):
                nc.sync.dma_start(W[:, 2 * EC + c, :], wv_r[:, c, :])
            yield
            for c in range(EC):
                nc.sync.dma_start(W[:, 3 * EC + c, :], wo_r[:, c, :])
            yield

        xT_sb: dict = {}
        labT_sb: dict = {}
        qT_sb: dict = {}
        kT_sb: dict = {}
        v_sb: dict = {}
        att_sb: dict = {}

        def g_qkv(b, sections):
            if "init" in sections:
                if b not in xT_sb:
                    xt = xtp.tile([P, EC, QL], BF16, tag="xT")
                    xr = xT_d[b].rearrange("(c p) t -> p c t", p=P)
                    for c in range(EC):
                        for qh in range(QH):
                            nc.sync.dma_start(xt[:, c, qh * QW:(qh + 1) * QW],
                                              xr[:, c, qh * QW:(qh + 1) * QW])
                    xT_sb[b] = xt
                yield
            if "k" in sections or "v" in sections:
                if b not in labT_sb:
                    lt = ltp.tile([P, EC, KL], BF16, tag="labT")
                    lr = labT_d[b].rearrange("(c p) t -> p c t", p=P)
                    for c in range(EC):
                        nc.sync.dma_start(lt[:, c, :], lr[:, c, :])
                    labT_sb[b] = lt
            if "q" in sections:
                qt_t = qtp.tile([P, MC, QL], BF16, tag="qT")
                qT_sb[b] = qt_t
                for m in range(MC):
                    for qh in range(QH):
                        pt = pp.tile([P, 512], F32, tag="pp")
                        for c in range(EC):
                            nc.tensor.matmul(
                                pt[:], W[:, c, m * P:(m + 1) * P],
                                xT_sb[b][:, c, qh * QW:(qh + 1) * QW],
                                start=(c == 0), stop=(c == EC - 1))
                        nc.vector.tensor_copy(qt_t[:, m, qh * QW:(qh + 1) * QW], pt[:])
                        yield
            if "k" in sections:
                kt_t = ktp.tile([P, MC, KL], BF16, tag="kT")
                kT_sb[b] = kt_t
                for m in range(MC):
                    pt = pp.tile([P, 512], F32, tag="pp")
                    for c in range(EC):
                        nc.tensor.matmul(
                            pt[:], W[:, EC + c, m * P:(m + 1) * P],
                            labT_sb[b][:, c, :],
                            start=(c == 0), stop=(c == EC - 1))
                    nc.vector.tensor_copy(kt_t[:, m, :], pt[:])
                    yield
            if "v" in sections:
                v_t = vtp.tile([P, KC, HEADS, DHEAD + 1], BF16, tag="v")
                v_sb[b] = v_t
                nc.vector.memset(v_t[:, :, :, DHEAD:DHEAD + 1], 1.0)
                for t in range(KC):
                    for n0, nw in ((0, 512), (512, 256)):
                        pt = pp.tile([P, 512], F32, tag="pp")
                        for c in range(EC):
                            nc.tensor.matmul(
                                pt[:, :nw], labT_sb[b][:, c, t * P:(t + 1) * P],
                                W[:, 2 * EC + c, n0:n0 + nw],
                                start=(c == 0), stop=(c == EC - 1))
                        h0, h1 = n0 // DHEAD, (n0 + nw) // DHEAD
                        nc.vector.tensor_copy(
                            v_t[:, t, h0:h1, 0:DHEAD],
                            pt[:, :nw].rearrange("p (h d) -> p h d", d=DHEAD))
                        yield

        def g_att(b):
            att_t = {qh: attp.tile([P, MC, QW], BF16, tag="att", name=f"att_{b}_{qh}")
                     for qh in range(QH)}
            for qh in range(QH):
                att_sb[(b, qh)] = att_t[qh]
            qt_t = qT_sb[b]
            kt_t = kT_sb[b]
            v_t = v_sb[b]
            for hc in range(HEADS // 2):
                pairsums = smp.tile([2, QH, QW], F32, tag="sums")
                ex = {par: expp.tile([P, KC, QL], BF16, tag="exp",
                                     name=f"ex_{b}_{hc}_{par}")
                      for par in range(2)}
                for kc in range(KC):
                    ss = {par: ps.tile([P, QL], F32, tag="ps",
                                       name=f"ss_{b}_{hc}_{kc}_{par}")
                          for par in range(2)}
                    # interleave the two heads of the pair: they sit on
                    # different PE row strips (0-63 / 64-127) and execute
                    # concurrently on the systolic array
                    for qh in range(QH):
                        for par in range(2):
                            p0 = par * DHEAD
                            nc.tensor.matmul(
                                ss[par][:, qh * QW:(qh + 1) * QW],
                                kt_t[p0:p0 + DHEAD, hc, kc * P:(kc + 1) * P],
                                qt_t[p0:p0 + DHEAD, hc, qh * QW:(qh + 1) * QW])
                    for par in range(2):
                        nc.scalar.activation(ex[par][:, kc, :], ss[par][:],
                                             mybir.ActivationFunctionType.Exp,
                                             bias=biask_sb[:, b, kc:kc + 1],
                                             scale=SCALE)
                yield
                for par in range(2):
                    h = 2 * hc + par
                    p0 = par * DHEAD
                    for qh in range(QH):
                        pa_t = pa.tile([DHEAD + 1, QW], F32, tag="pa")
                        for kc in range(KC):
                            nc.tensor.matmul(pa_t[:], v_t[:, kc, h, :],
                                             ex[par][:, kc, qh * QW:(qh + 1) * QW],
                                             start=(kc == 0), stop=(kc == KC - 1))
                        st_t = stp.tile([DHEAD + 1, QW], F32, tag="stage")
                        nc.vector.tensor_copy(st_t[:], pa_t[:])
                        nc.gpsimd.dma_start(att_t[qh][p0:p0 + DHEAD, hc, :],
                                            st_t[0:DHEAD, :])
                        nc.sync.dma_start(pairsums[par:par + 1, qh, :],
                                          st_t[DHEAD:DHEAD + 1, :])
                yield
                # normalize this head pair for both q-halves
                rec2 = rcp.tile([2, QH, QW], F32, tag="rec")
                nc.vector.reciprocal_approx_fast(rec2[:], pairsums[:])
                rec2b = rcp.tile([2, QH, QW], BF16, tag="recb")
                nc.vector.tensor_copy(rec2b[:], rec2[:])
                for qh in range(QH):
                    rb = rsp.tile([1, QW], BF16, tag="rstage")
                    nc.sync.dma_start(rb[:], rec2b[1:2, qh, :])
                    ba = bcp.tile([P, QW], BF16, tag="bc")
                    nc.gpsimd.partition_broadcast(ba[0:DHEAD, :], rec2b[0:1, qh, :])
                    bb = bcp.tile([P, QW], BF16, tag="bc")
                    nc.gpsimd.partition_broadcast(bb[:], rb[:])
                    a_t = att_t[qh]
                    nc.vector.tensor_mul(a_t[0:DHEAD, hc, :],
                                         a_t[0:DHEAD, hc, :], ba[0:DHEAD, :])
                    nc.vector.tensor_mul(a_t[DHEAD:P, hc, :],
                                         a_t[DHEAD:P, hc, :], bb[DHEAD:P, :])
                yield

        def g_out(b, qh, defer=False):
            parts = ((0, 512), (512, 256))

            def emit_head(att_t, qt, part, pool, name):
                n0, nw = parts[part]
                po = pool.tile([P, 512], F32, tag=pool.name.split("_")[0],
                               name=name)
                for c in range(MC - 1):
                    nc.tensor.matmul(po[:, :nw],
                                     att_t[:, c, qt * P:(qt + 1) * P],
                                     W[:, 3 * EC + c, n0:n0 + nw],
                                     start=(c == 0), stop=False)
                return po

            def emit_tail(att_t, qt, part, po, ou, xn):
                n0, nw = parts[part]
                c = MC - 1
                nc.tensor.matmul(po[:, :nw],
                                 att_t[:, c, qt * P:(qt + 1) * P],
                                 W[:, 3 * EC + c, n0:n0 + nw],
                                 start=False, stop=True)
                nc.vector.tensor_add(ou[:, n0:n0 + nw], po[:, :nw],
                                     xn[:, n0:n0 + nw])

            if not defer:
                att_t = att_sb[(b, qh)]
                for qt in range(QT):
                    qg = qh * QT + qt
                    xn = onp.tile([P, EMBED], F32, tag="xn")
                    nc.sync.dma_start(xn[:], x_d[b, qg * P:(qg + 1) * P, :])
                    ou = oup.tile([P, EMBED], F32, tag="ou")
                    for part in range(2):
                        po = emit_head(att_t, qt, part, pp, f"po_{b}_{qh}_{qt}_{part}")
                        emit_tail(att_t, qt, part, po, ou, xn)
                    nc.sync.dma_start(out_d[b, qg * P:(qg + 1) * P, :], ou[:])
                    yield
            else:
                # software pipeline over both q-halves: 4 psum groups open
                # (pp + idle ps banks) so norm-gated final-chunk matmuls
                # overlap useful work
                tiles = [(h, t) for h in range(QH) for t in range(QT)]
                pend = {}
                depth = 3
                pools = [ps, pp, pa]
                for i in range(depth):
                    h, t = tiles[i]
                    for part in range(2):
                        pend[(h, t, part)] = emit_head(
                            att_sb[(b, h)], t, part, pools[i % 3],
                            f"po_{b}_{h}_{t}_{part}")
                for i, (h, t) in enumerate(tiles):
                    qg = h * QT + t
                    xn = onp.tile([P, EMBED], F32, tag="xn")
                    nc.sync.dma_start(xn[:], x_d[b, qg * P:(qg + 1) * P, :])
                    ou = oup.tile([P, EMBED], F32, tag="ou")
                    for part in range(2):
                        emit_tail(att_sb[(b, h)], t, part,
                                  pend.pop((h, t, part)), ou, xn)
                    nc.sync.dma_start(out_d[b, qg * P:(qg + 1) * P, :], ou[:])
                    ni = i + depth
                    if ni < len(tiles):
                        nh, nt = tiles[ni]
                        for part in range(2):
                            pend[(nh, nt, part)] = emit_head(
                                att_sb[(b, nh)], nt, part, pools[ni % 3],
                                f"po_{b}_{nh}_{nt}_{part}")
                    yield

        def rr(*gens):
            rr_w([(g, 1) for g in gens])

        def chain(*gens):
            for g in gens:
                yield from g

        def rr_w(pairs):
            live = [[iter(g), w] for g, w in pairs]
            while live:
                for item in list(live):
                    g, w = item
                    for _ in range(w):
                        try:
                            next(g)
                        except StopIteration:
                            live.remove(item)
                            break

        rr(g_preload(), g_qkv(0, ("init", "q", "k")))
        rr(g_qkv(0, ("v",)))
        rr(g_att(0), g_qkv(1, ("init", "q", "k", "v")))
        rr_w([(g_att(1), 2), (chain(g_out(0, 0), g_out(0, 1)), 1)])
        rr(g_out(1, 0, defer=True))

    nc.compile()
    return nc


def _get_nc():
    if "nc" not in _CACHE:
        _CACHE["nc"] = _build()
    return _CACHE["nc"]


def _prep(inputs):
    x = np.asarray(inputs["image_embeddings"], dtype=np.float32)
    lab = np.asarray(inputs["lab_embeddings"], dtype=np.float32)
    lv = np.asarray(inputs["lab_values"], dtype=np.float32)
    Wq = np.asarray(inputs["Wq"], dtype=np.float32)
    Wk = np.asarray(inputs["Wk"], dtype=np.float32)
    Wv = np.asarray(inputs["Wv"], dtype=np.float32)
    Wo = np.asarray(inputs["Wo"], dtype=np.float32)
    bo = np.asarray(inputs["bo"], dtype=np.float32)
    table = np.asarray(inputs["bias_table"], dtype=np.float32)
    vp_w = np.asarray(inputs["vp_w"], dtype=np.float32)
    vp_b = np.asarray(inputs["vp_b"], dtype=np.float32)
    fus_w = np.asarray(inputs["fus_w"], dtype=np.float32)
    fus_b = np.asarray(inputs["fus_b"], dtype=np.float32)
    idx = np.asarray(inputs["lab_test_indices"])
    mask = np.asarray(inputs["mask"])

    # per-key additive bias: embedding + linear + tanh + clamp, then mask
    tb = table[idx, 0]                                   # [B, KL] f32
    vb = lv * vp_w[0, 0] + vp_b[0]
    tv = np.tanh(tb * fus_w[0, 0] + vb * fus_w[1, 0] + fus_b[0])
    tv = np.clip(tv, -5.0, 5.0).astype(np.float32)
    biasK = np.where(mask == 0, np.float32(-1e9), tv).astype(np.float32)

    xT = np.ascontiguousarray(x.transpose(0, 2, 1)).astype(BF)
    labT = np.ascontiguousarray(lab.transpose(0, 2, 1)).astype(BF)
    x_pb = x + bo  # fold output bias into the residual
    shared = {
        "Wq": Wq.astype(BF), "Wk": Wk.astype(BF), "Wv": Wv.astype(BF),
        "Wo": Wo.astype(BF),
    }
    in_maps = []
    for i in range(NCORES):
        s = slice(BLOC * i, BLOC * (i + 1))
        in_maps.append({
            "xT": xT[s], "labT": labT[s],
            "x": np.ascontiguousarray(x_pb[s]),
            "biasK": np.ascontiguousarray(biasK[s]),
            **shared,
        })
    return in_maps


def run(inputs, trace=False, tmpdir=None):
    nc = _get_nc()
    in_maps = _prep(inputs)
    res = bass_utils.run_bass_kernel_spmd(
        nc, in_maps, core_ids=list(range(NCORES)), trace=trace, tmpdir=tmpdir)
    out = np.concatenate([res.results[i]["out"] for i in range(NCORES)], axis=0)
    return out, res


def kernel(**inputs) -> np.ndarray:
    out, _ = run(inputs)
    return out


if __name__ == "__main__":
    rng = np.random.default_rng(0)
    fake = {
        "image_embeddings": rng.standard_normal((B, QL, EMBED)).astype(np.float32),
        "lab_embeddings": rng.standard_normal((B, KL, EMBED)).astype(np.float32),
        "lab_values": rng.standard_normal((B, KL)).astype(np.float32),
        "Wq": (rng.standard_normal((EMBED, INNER)) * 0.02).astype(np.float32),
        "Wk": (rng.standard_normal((EMBED, INNER)) * 0.02).astype(np.float32),
        "Wv": (rng.standard_normal((EMBED, INNER)) * 0.02).astype(np.float32),
        "Wo": (rng.standard_normal((INNER, EMBED)) * 0.02).astype(np.float32),
        "bo": np.zeros(EMBED, np.float32),
        "bias_table": (rng.standard_normal((1001, 1)) * 0.02).astype(np.float32),
        "vp_w": rng.standard_normal((1, 1)).astype(np.float32),
        "vp_b": np.zeros(1, np.float32),
        "fus_w": rng.standard_normal((2, 1)).astype(np.float32),
        "fus_b": np.zeros(1, np.float32),
        "lab_test_indices": rng.integers(0, 1001, (B, KL)),
        "mask": rng.integers(0, 2, (B, KL)).astype(np.int32),
    }
    out = kernel(**fake)
    print("out", out.shape, out.dtype, float(np.abs(out).max()))

